# revision 11
# baseline (speedup 1.0000x reference)
"""Embedding lookup (gather rows of W.T by index, + bias) on 8 TRN2 cores.

Strategy: vocab-sharded ("row-parallel") embedding. The bias is folded into
the table on the host (out = (W.T + b)[x], exactly). Each core owns a
12500-row shard of the 100000-row table; the host routes each token index to
its owning core (a sort by index value, which both groups tokens by shard and
orders gather reads for HBM row locality), the device does the entire data
movement: an indexed-gather DMA of 256 B rows from HBM -> SBUF, then streams
the gathered rows back out to HBM. The host applies the inverse routing
permutation to assemble the full [4096, 200, 64] output.

Device kernel (SPMD on 8 cores, identical program):
  - idx tile [128, N_PAD/16] int16 loaded to SBUF once (dma_gather expects
    index i of a chunk at partition i%16, free slot i//16, replicated on all
    8 16-partition groups).
  - NCH chunks of C indices (single_packet=False: the 64-desc/lane packet
    cap only applies to coalesced packets). Chunk size amortizes the ~1 us
    fixed + ~1.3 ns/idx gpsimd engine cost per dma_gather (the binding
    resource at small chunks). Chunks rotate over 4 SWDGE queues and NB SBUF
    buffers; the two HWDGE engines (sync/scalar) stream gathered buffers to
    the output HBM tensor, overlapped with subsequent gathers.
"""

import numpy as np

import concourse.bass as bass
import concourse.bacc as bacc
import concourse.mybir as mybir
from concourse.library_config import mlp
from concourse.bass_utils import run_bass_kernel_spmd

VOCAB = 100000
E = 64                  # embedding dim; 64 * 4 B = 256 B rows
N_CORES = 8
SHARD = VOCAB // N_CORES            # 12500 rows per core (< int16 max)
C = 4096                # indices per dma_gather
NCH = 26                # chunks per core
N_PAD = C * NCH         # padded indices per core (>= max bucket 102771)
F = C // 128            # free slots per chunk in the SBUF gather buffer
NB = 6                  # rotating gather buffers
NQ = 4                  # SWDGE queues (one descriptor ring each)
CS = C // 16            # idx-tile columns per chunk
# idx tile loads in 4 pieces (chunk bounds), overlapping the library load
IDX_PIECES = [0, (NCH + 3) // 4, (NCH + 1) // 2, (3 * NCH) // 4, NCH]

_compiled = None


def _build():
    nc = bacc.Bacc("TRN2", num_swdge_queues=NQ)
    w_hbm = nc.dram_tensor("w", [SHARD, E], mybir.dt.float32, kind="ExternalInput")
    idx_hbm = nc.dram_tensor(
        "idx", [128, N_PAD // 16], mybir.dt.int16, kind="ExternalInput"
    )
    out_hbm = nc.dram_tensor(
        "out", [NCH, 128, F * E], mybir.dt.float32, kind="ExternalOutput"
    )

    import contextlib

    with contextlib.ExitStack() as stack:
        block = stack.enter_context(nc.Block())
        idxs_sbuf = stack.enter_context(
            nc.sbuf_tensor("idxs_sbuf", [128, N_PAD // 16], mybir.dt.int16)
        )
        bufs = [
            stack.enter_context(
                nc.sbuf_tensor(f"buf{j}", [128, F, E], mybir.dt.float32)
            )
            for j in range(NB)
        ]
        isems = [stack.enter_context(nc.semaphore(f"isem{p}")) for p in range(4)]
        gsems = [stack.enter_context(nc.semaphore(f"g{j}")) for j in range(NB)]
        wsems = [stack.enter_context(nc.semaphore(f"ws{j}")) for j in range(NB)]

        @block.gpsimd
        def _(g: bass.BassGpSimd):
            # idx pieces via SWDGE (deterministic +16/DMA); drains overlap
            # the library load that follows
            for p in range(4):
                a, b = IDX_PIECES[p] * CS, IDX_PIECES[p + 1] * CS
                g.dma_start(idxs_sbuf[:, a:b], idx_hbm[:, a:b]).then_inc(
                    isems[p], 16
                )
            g.load_library(mlp)
            for k in range(NCH):
                j = k % NB
                if k in IDX_PIECES:
                    p = IDX_PIECES.index(k)
                    g.wait_ge(isems[p], 16)
                if k >= NB:
                    # WAR: wait for write-out of the chunk that last used slot j
                    g.wait_ge(wsems[j], 16 * ((k - NB) // NB + 1))
                g.dma_gather(
                    bufs[j][:],
                    w_hbm[:],
                    idxs_sbuf[:, k * CS : (k + 1) * CS],
                    C,
                    C,
                    E,
                    queue_num=k % NQ,
                    single_packet=False,
                ).then_inc(gsems[j], 16)

        # write-outs split across the two HWDGE engines (SP=even, ACT=odd
        # chunks) to double issue streams and reduce head-of-line blocking
        def _writer(eng, parity):
            for k in range(parity, NCH, 2):
                j = k % NB
                eng.wait_ge(gsems[j], 16 * (k // NB + 1))
                eng.dma_start(out_hbm[k], bufs[j][:]).then_inc(wsems[j], 16)
            for j in range(parity, NB, 2):
                ks = [k for k in range(NCH) if k % NB == j]
                eng.wait_ge(wsems[j], 16 * len(ks))

        @block.sync
        def _(s: bass.BassEngine):
            _writer(s, 0)

        @block.scalar
        def _(sc: bass.BassEngine):
            _writer(sc, 1)

    nc.compile()
    return nc


def _get_compiled():
    global _compiled
    if _compiled is None:
        _compiled = _build()
    return _compiled


def _run(x, W, b, trace=False):
    x = np.asarray(x)
    W = np.asarray(W, dtype=np.float32)
    b = np.asarray(b, dtype=np.float32)
    orig_shape = x.shape
    xf = np.ascontiguousarray(x).reshape(-1).astype(np.int64)
    n_tok = xf.shape[0]

    table = W.T + b  # [VOCAB, E]; bias folded in exactly (fp32 add, matches ref)

    # route tokens: sort by index value == group by owning core + ascending
    # rows within each core (sequential-ish HBM reads)
    order = np.argsort(xf, kind="stable")
    sx = xf[order]
    counts = np.bincount(sx // SHARD, minlength=N_CORES)
    starts = np.concatenate(([0], np.cumsum(counts)))[:N_CORES]

    in_maps = []
    overflow = []  # (core, positions) handled on host if a bucket > N_PAD
    for c in range(N_CORES):
        n_c = int(counts[c])
        pos_c = order[starts[c] : starts[c] + n_c]
        if n_c > N_PAD:
            overflow.append(pos_c[N_PAD:])
            pos_c = pos_c[:N_PAD]
            n_c = N_PAD
        loc = (xf[pos_c] - c * SHARD).astype(np.int16)
        pad = np.zeros(N_PAD, dtype=np.int16)
        pad[:n_c] = loc
        # chunk k, index i -> partition i%16, column k*CS + i//16; replicate x8
        tile16 = pad.reshape(NCH, CS, 16).transpose(2, 0, 1).reshape(16, -1)
        idx_tile = np.tile(tile16, (8, 1))
        w_shard = np.ascontiguousarray(table[c * SHARD : (c + 1) * SHARD])
        in_maps.append({"w": w_shard, "idx": idx_tile})

    nc = _get_compiled()
    br = run_bass_kernel_spmd(
        nc, in_maps, core_ids=list(range(N_CORES)), trace=trace
    )

    out_full = np.empty((n_tok, E), dtype=np.float32)
    for c in range(N_CORES):
        n_c = min(int(counts[c]), N_PAD)
        pos_c = order[starts[c] : starts[c] + n_c]
        dev = br.results[c]["out"].reshape(NCH, 128, F, E)
        # gathered row i of chunk k lives at [k, i%128, i//128]
        rows = dev.transpose(0, 2, 1, 3).reshape(N_PAD, E)
        out_full[pos_c] = rows[:n_c]
    for pos in overflow:  # statistically never taken; exact host fallback
        out_full[pos] = table[xf[pos]]

    return out_full.reshape(*orig_shape, E), br


def kernel(x, W, b):
    out, _ = _run(x, W, b, trace=False)
    return out


# revision 12
# speedup vs baseline: 1.1350x; 1.1350x over previous
"""Embedding lookup (gather rows of W.T by index, + bias) on 8 TRN2 cores.

Strategy: vocab-sharded ("row-parallel") embedding. The bias is folded into
the table on the host (out = (W.T + b)[x], exactly). Each core owns a
12500-row shard of the 100000-row table; the host routes each token index to
its owning core (a sort by index value, which both groups tokens by shard and
orders gather reads for HBM row locality), the device does the entire data
movement: an indexed-gather DMA of 256 B rows from HBM -> SBUF, then streams
the gathered rows back out to HBM. The host applies the inverse routing
permutation to assemble the full [4096, 200, 64] output.

Device kernel (SPMD on 8 cores, identical program):
  - idx tile [128, N_PAD/16] int16 loaded to SBUF once (dma_gather expects
    index i of a chunk at partition i%16, free slot i//16, replicated on all
    8 16-partition groups).
  - NCH chunks of C indices (single_packet=False: the 64-desc/lane packet
    cap only applies to coalesced packets). Chunk size amortizes the ~1 us
    fixed + ~1.3 ns/idx gpsimd engine cost per dma_gather (the binding
    resource at small chunks). Chunks rotate over 4 SWDGE queues and NB SBUF
    buffers; the two HWDGE engines (sync/scalar) stream gathered buffers to
    the output HBM tensor, overlapped with subsequent gathers.
"""

import numpy as np

import concourse.bass as bass
import concourse.bacc as bacc
import concourse.mybir as mybir
from concourse.library_config import mlp
from concourse.bass_utils import run_bass_kernel_spmd

VOCAB = 100000
E = 64                  # embedding dim; 64 * 4 B = 256 B rows
N_CORES = 8
SHARD = VOCAB // N_CORES            # 12500 rows per core (< int16 max)
C = 1024                # indices per dma_gather
NCH = 102               # chunks per core
N_PAD = C * NCH         # padded indices per core (>= max bucket 102771)
F = C // 128            # free slots per chunk in the SBUF gather buffer
NB = 24                 # rotating gather buffers
NQ = 4                  # SWDGE queues (one descriptor ring each)
CS = C // 16            # idx-tile columns per chunk
# idx tile loads in 4 pieces (chunk bounds); small first piece so gathers
# start early, the rest stream in behind
IDX_PIECES = [0, 4, 37, 70, NCH]

_compiled = None


def _build():
    nc = bacc.Bacc("TRN2", num_swdge_queues=NQ)
    w_hbm = nc.dram_tensor("w", [SHARD, E], mybir.dt.float32, kind="ExternalInput")
    idx_hbm = nc.dram_tensor(
        "idx", [128, N_PAD // 16], mybir.dt.int16, kind="ExternalInput"
    )
    out_hbm = nc.dram_tensor(
        "out", [NCH, 128, F * E], mybir.dt.float32, kind="ExternalOutput"
    )

    import contextlib

    with contextlib.ExitStack() as stack:
        block = stack.enter_context(nc.Block())
        idxs_sbuf = stack.enter_context(
            nc.sbuf_tensor("idxs_sbuf", [128, N_PAD // 16], mybir.dt.int16)
        )
        bufs = [
            stack.enter_context(
                nc.sbuf_tensor(f"buf{j}", [128, F, E], mybir.dt.float32)
            )
            for j in range(NB)
        ]
        isems = [stack.enter_context(nc.semaphore(f"isem{p}")) for p in range(4)]
        gsems = [stack.enter_context(nc.semaphore(f"g{j}")) for j in range(NB)]
        wsems = [stack.enter_context(nc.semaphore(f"ws{j}")) for j in range(NB)]

        @block.gpsimd
        def _(g: bass.BassGpSimd):
            # idx pieces via SWDGE (deterministic +16/DMA); drains overlap
            # the library load that follows
            for p in range(4):
                a, b = IDX_PIECES[p] * CS, IDX_PIECES[p + 1] * CS
                g.dma_start(idxs_sbuf[:, a:b], idx_hbm[:, a:b]).then_inc(
                    isems[p], 16
                )
            g.load_library(mlp)
            for k in range(NCH):
                j = k % NB
                if k in IDX_PIECES:
                    p = IDX_PIECES.index(k)
                    g.wait_ge(isems[p], 16)
                if k >= NB:
                    # WAR: wait for write-out of the chunk that last used slot j
                    g.wait_ge(wsems[j], 16 * ((k - NB) // NB + 1))
                g.dma_gather(
                    bufs[j][:],
                    w_hbm[:],
                    idxs_sbuf[:, k * CS : (k + 1) * CS],
                    C,
                    C,
                    E,
                    queue_num=k % NQ,
                ).then_inc(gsems[j], 16)

        # write-outs split across the two HWDGE engines (SP=even, ACT=odd
        # chunks) to double issue streams and reduce head-of-line blocking
        def _writer(eng, parity):
            for k in range(parity, NCH, 2):
                j = k % NB
                eng.wait_ge(gsems[j], 16 * (k // NB + 1))
                eng.dma_start(out_hbm[k], bufs[j][:]).then_inc(wsems[j], 16)
            for j in range(parity, NB, 2):
                ks = [k for k in range(NCH) if k % NB == j]
                eng.wait_ge(wsems[j], 16 * len(ks))

        @block.sync
        def _(s: bass.BassEngine):
            _writer(s, 0)

        @block.scalar
        def _(sc: bass.BassEngine):
            _writer(sc, 1)

    nc.compile()
    return nc


def _get_compiled():
    global _compiled
    if _compiled is None:
        _compiled = _build()
    return _compiled


def _run(x, W, b, trace=False):
    x = np.asarray(x)
    W = np.asarray(W, dtype=np.float32)
    b = np.asarray(b, dtype=np.float32)
    orig_shape = x.shape
    xf = np.ascontiguousarray(x).reshape(-1).astype(np.int64)
    n_tok = xf.shape[0]

    table = W.T + b  # [VOCAB, E]; bias folded in exactly (fp32 add, matches ref)

    # route tokens: sort by index value == group by owning core + ascending
    # rows within each core (sequential-ish HBM reads)
    order = np.argsort(xf, kind="stable")
    sx = xf[order]
    counts = np.bincount(sx // SHARD, minlength=N_CORES)
    starts = np.concatenate(([0], np.cumsum(counts)))[:N_CORES]

    in_maps = []
    overflow = []  # (core, positions) handled on host if a bucket > N_PAD
    for c in range(N_CORES):
        n_c = int(counts[c])
        pos_c = order[starts[c] : starts[c] + n_c]
        if n_c > N_PAD:
            overflow.append(pos_c[N_PAD:])
            pos_c = pos_c[:N_PAD]
            n_c = N_PAD
        loc = (xf[pos_c] - c * SHARD).astype(np.int16)
        pad = np.zeros(N_PAD, dtype=np.int16)
        pad[:n_c] = loc
        # chunk k, index i -> partition i%16, column k*CS + i//16; replicate x8
        tile16 = pad.reshape(NCH, CS, 16).transpose(2, 0, 1).reshape(16, -1)
        idx_tile = np.tile(tile16, (8, 1))
        w_shard = np.ascontiguousarray(table[c * SHARD : (c + 1) * SHARD])
        in_maps.append({"w": w_shard, "idx": idx_tile})

    nc = _get_compiled()
    br = run_bass_kernel_spmd(
        nc, in_maps, core_ids=list(range(N_CORES)), trace=trace
    )

    out_full = np.empty((n_tok, E), dtype=np.float32)
    for c in range(N_CORES):
        n_c = min(int(counts[c]), N_PAD)
        pos_c = order[starts[c] : starts[c] + n_c]
        dev = br.results[c]["out"].reshape(NCH, 128, F, E)
        # gathered row i of chunk k lives at [k, i%128, i//128]
        rows = dev.transpose(0, 2, 1, 3).reshape(N_PAD, E)
        out_full[pos_c] = rows[:n_c]
    for pos in overflow:  # statistically never taken; exact host fallback
        out_full[pos] = table[xf[pos]]

    return out_full.reshape(*orig_shape, E), br


def kernel(x, W, b):
    out, _ = _run(x, W, b, trace=False)
    return out


# revision 15
# speedup vs baseline: 1.2029x; 1.0598x over previous
"""Embedding lookup (gather rows of W.T by index, + bias) on 8 TRN2 cores.

Strategy: vocab-sharded ("row-parallel") embedding. The bias is folded into
the table on the host (out = (W.T + b)[x], exactly). Each core owns a
12500-row shard of the 100000-row table; the host routes each token index to
its owning core via one argsort (grouping by shard AND sorting ascending
within it), the device does the data movement, and the host applies the
inverse permutation to assemble the full [4096, 200, 64] output.

Device kernel (SPMD on 8 cores, identical program), built around the
gpsimd dma_gather primitive (SWDGE: one DMA descriptor per index):

- QUAD pass: sorted indices have ~8x multiplicity, so 4 consecutive sorted
  tokens almost always fall within a 4-row window of the table. One 1024 B
  descriptor (4 overlapping rows, elem_step=64 elems, elem_size=256) serves
  4 tokens, amortizing the ~200 ns HBM random-read latency per descriptor
  that dominates at 256 B. The host picks each quad's base row and later
  slices each token's row out of its quad block (pure permutation).
- SINGLES pass: the rare tokens whose row falls outside their quad's 4-row
  window (~0.1% at this multiplicity) are gathered separately at 256 B.
- Chunks of 1024 indices (single_packet dma_gather caps at 64 descs/lane),
  rotating over 4 SWDGE queues (one descriptor ring each) and 8 SBUF
  buffers; the two HWDGE engines (sync/scalar) stream gathered buffers to
  HBM, overlapped with subsequent gathers.
"""

import contextlib

import numpy as np

import concourse.bass as bass
import concourse.bacc as bacc
import concourse.mybir as mybir
from concourse.library_config import mlp
from concourse.bass_utils import run_bass_kernel_spmd

VOCAB = 100000
E = 64                    # embedding dim; 256 B rows
QE = 4 * E                # quad block: 4 rows = 1024 B
N_CORES = 8
SHARD = VOCAB // N_CORES  # 12500 rows per core (< int16 max)
C = 1024                  # indices per dma_gather (single_packet limit)
N_PAD = 104448            # padded tokens per core (max bucket 102771 @ seed)
N_QUAD = N_PAD // 4       # 26112 quads
QCH = 26                  # quad chunks (26624 quad slots, tail padded)
SCH = 4                   # singles chunks (4096 slots for quad violators)
F = C // 128              # free slots per chunk
NB = 8                    # rotating quad buffers
NBS = 2                   # rotating singles buffers
NQ = 4                    # SWDGE queues
CS = C // 16              # idx-tile columns per chunk
QIDX_PIECES = [0, 4, 15, QCH]  # quad idx tile load pieces (chunk bounds)

_compiled = None


def _build():
    nc = bacc.Bacc("TRN2", num_swdge_queues=NQ)
    w_hbm = nc.dram_tensor("w", [SHARD, E], mybir.dt.float32, kind="ExternalInput")
    qidx_hbm = nc.dram_tensor(
        "qidx", [128, QCH * CS], mybir.dt.int16, kind="ExternalInput"
    )
    sidx_hbm = nc.dram_tensor(
        "sidx", [128, SCH * CS], mybir.dt.int16, kind="ExternalInput"
    )
    outq_hbm = nc.dram_tensor(
        "outq", [QCH, 128, F * QE], mybir.dt.float32, kind="ExternalOutput"
    )
    outs_hbm = nc.dram_tensor(
        "outs", [SCH, 128, F * E], mybir.dt.float32, kind="ExternalOutput"
    )

    # overlapping view of the table: "row" r = elements [r*64, r*64 + 256)
    w_quad = w_hbm[:].copy()
    w_quad.ap[0] = (E, SHARD - 3)
    w_quad.ap[1] = (1, QE)

    with contextlib.ExitStack() as stack:
        block = stack.enter_context(nc.Block())
        qidx_sb = stack.enter_context(
            nc.sbuf_tensor("qidx_sb", [128, QCH * CS], mybir.dt.int16)
        )
        sidx_sb = stack.enter_context(
            nc.sbuf_tensor("sidx_sb", [128, SCH * CS], mybir.dt.int16)
        )
        qbufs = [
            stack.enter_context(
                nc.sbuf_tensor(f"qbuf{j}", [128, F, QE], mybir.dt.float32)
            )
            for j in range(NB)
        ]
        sbufs = [
            stack.enter_context(
                nc.sbuf_tensor(f"sbuf{j}", [128, F, E], mybir.dt.float32)
            )
            for j in range(NBS)
        ]
        isems = [stack.enter_context(nc.semaphore(f"isem{p}")) for p in range(3)]
        ssem = stack.enter_context(nc.semaphore("ssem"))
        gsems = [stack.enter_context(nc.semaphore(f"g{j}")) for j in range(NB)]
        wsems = [stack.enter_context(nc.semaphore(f"ws{j}")) for j in range(NB)]
        gsems_s = [stack.enter_context(nc.semaphore(f"gs{j}")) for j in range(NBS)]
        wsems_s = [stack.enter_context(nc.semaphore(f"wss{j}")) for j in range(NBS)]

        @block.gpsimd
        def _(g: bass.BassGpSimd):
            # idx loads via SWDGE (deterministic +16/DMA); drains overlap the
            # library load that follows
            for p in range(3):
                a, b = QIDX_PIECES[p] * CS, QIDX_PIECES[p + 1] * CS
                g.dma_start(qidx_sb[:, a:b], qidx_hbm[:, a:b]).then_inc(isems[p], 16)
            g.dma_start(sidx_sb[:], sidx_hbm[:]).then_inc(ssem, 16)
            g.load_library(mlp)
            for k in range(QCH):
                j = k % NB
                if k in QIDX_PIECES[:3]:
                    g.wait_ge(isems[QIDX_PIECES.index(k)], 16)
                if k >= NB:
                    g.wait_ge(wsems[j], 16 * ((k - NB) // NB + 1))
                g.dma_gather(
                    qbufs[j][:],
                    w_quad,
                    qidx_sb[:, k * CS : (k + 1) * CS],
                    C,
                    C,
                    QE,
                    elem_step=E,
                    queue_num=k % NQ,
                ).then_inc(gsems[j], 16)
            g.wait_ge(ssem, 16)
            for k in range(SCH):
                j = k % NBS
                if k >= NBS:
                    g.wait_ge(wsems_s[j], 16 * ((k - NBS) // NBS + 1))
                g.dma_gather(
                    sbufs[j][:],
                    w_hbm[:],
                    sidx_sb[:, k * CS : (k + 1) * CS],
                    C,
                    C,
                    E,
                    queue_num=j,  # SWDGE completion sems are queue-locked
                ).then_inc(gsems_s[j], 16)

        # quad write-outs split across the two HWDGE engines (SP=even,
        # ACT=odd chunks); singles land on SP at the end
        def _writer(eng, parity):
            for k in range(parity, QCH, 2):
                j = k % NB
                eng.wait_ge(gsems[j], 16 * (k // NB + 1))
                eng.dma_start(outq_hbm[k], qbufs[j][:]).then_inc(wsems[j], 16)
            for j in range(parity, NB, 2):
                ks = [k for k in range(QCH) if k % NB == j]
                eng.wait_ge(wsems[j], 16 * len(ks))

        @block.sync
        def _(s: bass.BassEngine):
            _writer(s, 0)
            for k in range(SCH):
                j = k % NBS
                s.wait_ge(gsems_s[j], 16 * (k // NBS + 1))
                s.dma_start(outs_hbm[k], sbufs[j][:]).then_inc(wsems_s[j], 16)
            for j in range(NBS):
                ks = [k for k in range(SCH) if k % NBS == j]
                s.wait_ge(wsems_s[j], 16 * len(ks))

        @block.scalar
        def _(sc: bass.BassEngine):
            _writer(sc, 1)

    nc.compile()
    return nc


def _get_compiled():
    global _compiled
    if _compiled is None:
        _compiled = _build()
    return _compiled


def _idx_tile(vals, nch):
    """[nch*C] int16 -> dma_gather layout [128, nch*CS] (i -> partition i%16,
    col chunk*CS + i//16, replicated on the 8 partition groups)."""
    t = vals.reshape(nch, CS, 16).transpose(2, 0, 1).reshape(16, -1)
    return np.tile(t, (8, 1))


def _run(x, W, b, trace=False):
    x = np.asarray(x)
    W = np.asarray(W, dtype=np.float32)
    b = np.asarray(b, dtype=np.float32)
    orig_shape = x.shape
    xf = np.ascontiguousarray(x).reshape(-1).astype(np.int64)
    n_tok = xf.shape[0]

    table = W.T + b  # bias folded in exactly (fp32 add, matches reference)

    order = np.argsort(xf, kind="stable")
    counts = np.bincount(xf[order] // SHARD, minlength=N_CORES)
    starts = np.concatenate(([0], np.cumsum(counts)))[:N_CORES]

    in_maps = []
    host_jobs = []
    for c in range(N_CORES):
        n_c = int(counts[c])
        pos_c = order[starts[c] : starts[c] + n_c]
        extra_pos = None
        if n_c > N_PAD:  # statistically never; exact host fallback
            extra_pos = pos_c[N_PAD:]
            pos_c = pos_c[:N_PAD]
            n_c = N_PAD
        loc = (xf[pos_c] - c * SHARD).astype(np.int32)
        pad = np.full(N_PAD, loc[-1] if n_c else 0, dtype=np.int32)
        pad[:n_c] = loc  # tail padding keeps the array sorted

        base = np.minimum(pad[0::4], SHARD - 4)
        sub = pad.reshape(-1, 4) - base[:, None]
        ok = (sub >= 0) & (sub <= 3)
        left_j = np.flatnonzero(~ok.reshape(-1))  # token slots needing singles
        left_j = left_j[left_j < n_c]

        qvals = np.zeros(QCH * C, dtype=np.int16)
        qvals[:N_QUAD] = base.astype(np.int16)
        svals = np.zeros(SCH * C, dtype=np.int16)
        ns = min(len(left_j), SCH * C)
        svals[:ns] = pad[left_j[:ns]].astype(np.int16)

        in_maps.append(
            {
                "w": np.ascontiguousarray(table[c * SHARD : (c + 1) * SHARD]),
                "qidx": _idx_tile(qvals, QCH),
                "sidx": _idx_tile(svals, SCH),
            }
        )
        host_jobs.append((pos_c, n_c, sub, left_j, ns, extra_pos))

    nc = _get_compiled()
    br = run_bass_kernel_spmd(nc, in_maps, core_ids=list(range(N_CORES)), trace=trace)

    out_full = np.empty((n_tok, E), dtype=np.float32)
    tok_quad = np.arange(N_PAD) // 4
    for c in range(N_CORES):
        pos_c, n_c, sub, left_j, ns, extra_pos = host_jobs[c]
        # quad block i -> [chunk i//1024, partition i%128, slot (i%1024)//128]
        qdev = (
            br.results[c]["outq"]
            .reshape(QCH, 128, F, QE)
            .transpose(0, 2, 1, 3)
            .reshape(QCH * C, 4, E)
        )
        subf = np.clip(sub.reshape(-1), 0, 3)
        rows = qdev[tok_quad, subf]  # [N_PAD, E]
        if ns:
            sdev = (
                br.results[c]["outs"]
                .reshape(SCH, 128, F, E)
                .transpose(0, 2, 1, 3)
                .reshape(SCH * C, E)
            )
            rows[left_j[:ns]] = sdev[:ns]
        if len(left_j) > ns:  # singles overflow: exact host fallback
            j = left_j[ns:]
            rows[j] = table[xf[pos_c[j]]]
        out_full[pos_c] = rows[:n_c]
        if extra_pos is not None:
            out_full[extra_pos] = table[xf[extra_pos]]

    return out_full.reshape(*orig_shape, E), br


def kernel(x, W, b):
    out, _ = _run(x, W, b, trace=False)
    return out


# revision 16
# speedup vs baseline: 1.2547x; 1.0430x over previous
"""Embedding lookup (gather rows of W.T by index, + bias) on 8 TRN2 cores.

Strategy: vocab-sharded ("row-parallel") embedding. The bias is folded into
the table on the host (out = (W.T + b)[x], exactly). Each core owns a
12500-row shard of the 100000-row table; the host routes each token index to
its owning core via one argsort (grouping by shard AND sorting ascending
within it), the device does the data movement, and the host applies the
inverse permutation to assemble the full [4096, 200, 64] output.

Device kernel (SPMD on 8 cores, identical program), built around the
gpsimd dma_gather primitive (SWDGE: one DMA descriptor per index):

- BLOCK pass: sorted indices have ~8x multiplicity, so BLK=8 consecutive
  sorted tokens almost always fall within an 8-row window of the table. One
  2048 B descriptor (8 overlapping rows, elem_step=64 elems, elem_size=512)
  serves 8 tokens at SDMA line rate, amortizing the ~200 ns HBM random-read
  latency per descriptor that dominates at 256 B. The host picks each
  block's base row and later slices each token's row out of its block (pure
  permutation).
- SINGLES pass: the rare tokens whose row falls outside their block's 8-row
  window (none at this multiplicity, but kept for robustness) are gathered
  separately at 256 B.
- Chunks of 1024 indices (single_packet dma_gather caps at 64 descs/lane),
  rotating over 4 SWDGE queues (one descriptor ring each) and 8 SBUF
  buffers; the two HWDGE engines (sync/scalar) stream gathered buffers to
  HBM, overlapped with subsequent gathers.
"""

import contextlib

import numpy as np

import concourse.bass as bass
import concourse.bacc as bacc
import concourse.mybir as mybir
from concourse.library_config import mlp
from concourse.bass_utils import run_bass_kernel_spmd

VOCAB = 100000
E = 64                    # embedding dim; 256 B rows
BLK = 8                   # tokens (and table rows) per gathered block
QE = BLK * E              # block: 8 rows = 2048 B
N_CORES = 8
SHARD = VOCAB // N_CORES  # 12500 rows per core (< int16 max)
C = 1024                  # indices per dma_gather (single_packet limit)
N_PAD = 104448            # padded tokens per core (max bucket 102771 @ seed)
N_QUAD = N_PAD // BLK     # 13056 blocks
QCH = 13                  # block chunks (13312 slots, tail padded)
SCH = 4                   # singles chunks (4096 slots for quad violators)
F = C // 128              # free slots per chunk
NB = 8                    # rotating quad buffers
NBS = 2                   # rotating singles buffers
NQ = 4                    # SWDGE queues
CS = C // 16              # idx-tile columns per chunk
QIDX_PIECES = [0, 2, 7, QCH]  # block idx tile load pieces (chunk bounds)

_compiled = None


def _build():
    nc = bacc.Bacc("TRN2", num_swdge_queues=NQ)
    w_hbm = nc.dram_tensor("w", [SHARD, E], mybir.dt.float32, kind="ExternalInput")
    qidx_hbm = nc.dram_tensor(
        "qidx", [128, QCH * CS], mybir.dt.int16, kind="ExternalInput"
    )
    sidx_hbm = nc.dram_tensor(
        "sidx", [128, SCH * CS], mybir.dt.int16, kind="ExternalInput"
    )
    outq_hbm = nc.dram_tensor(
        "outq", [QCH, 128, F * QE], mybir.dt.float32, kind="ExternalOutput"
    )
    outs_hbm = nc.dram_tensor(
        "outs", [SCH, 128, F * E], mybir.dt.float32, kind="ExternalOutput"
    )

    # overlapping view of the table: "row" r = elements [r*64, r*64 + 256)
    w_quad = w_hbm[:].copy()
    w_quad.ap[0] = (E, SHARD - (BLK - 1))
    w_quad.ap[1] = (1, QE)

    with contextlib.ExitStack() as stack:
        block = stack.enter_context(nc.Block())
        qidx_sb = stack.enter_context(
            nc.sbuf_tensor("qidx_sb", [128, QCH * CS], mybir.dt.int16)
        )
        sidx_sb = stack.enter_context(
            nc.sbuf_tensor("sidx_sb", [128, SCH * CS], mybir.dt.int16)
        )
        qbufs = [
            stack.enter_context(
                nc.sbuf_tensor(f"qbuf{j}", [128, F, QE], mybir.dt.float32)
            )
            for j in range(NB)
        ]
        sbufs = [
            stack.enter_context(
                nc.sbuf_tensor(f"sbuf{j}", [128, F, E], mybir.dt.float32)
            )
            for j in range(NBS)
        ]
        isems = [stack.enter_context(nc.semaphore(f"isem{p}")) for p in range(3)]
        ssem = stack.enter_context(nc.semaphore("ssem"))
        gsems = [stack.enter_context(nc.semaphore(f"g{j}")) for j in range(NB)]
        wsems = [stack.enter_context(nc.semaphore(f"ws{j}")) for j in range(NB)]
        gsems_s = [stack.enter_context(nc.semaphore(f"gs{j}")) for j in range(NBS)]
        wsems_s = [stack.enter_context(nc.semaphore(f"wss{j}")) for j in range(NBS)]

        @block.gpsimd
        def _(g: bass.BassGpSimd):
            # idx loads via SWDGE (deterministic +16/DMA); drains overlap the
            # library load that follows
            for p in range(3):
                a, b = QIDX_PIECES[p] * CS, QIDX_PIECES[p + 1] * CS
                g.dma_start(qidx_sb[:, a:b], qidx_hbm[:, a:b]).then_inc(isems[p], 16)
            g.dma_start(sidx_sb[:], sidx_hbm[:]).then_inc(ssem, 16)
            g.load_library(mlp)
            for k in range(QCH):
                j = k % NB
                if k in QIDX_PIECES[:3]:
                    g.wait_ge(isems[QIDX_PIECES.index(k)], 16)
                if k >= NB:
                    g.wait_ge(wsems[j], 16 * ((k - NB) // NB + 1))
                g.dma_gather(
                    qbufs[j][:],
                    w_quad,
                    qidx_sb[:, k * CS : (k + 1) * CS],
                    C,
                    C,
                    QE,
                    elem_step=E,
                    queue_num=k % NQ,
                ).then_inc(gsems[j], 16)
            g.wait_ge(ssem, 16)
            for k in range(SCH):
                j = k % NBS
                if k >= NBS:
                    g.wait_ge(wsems_s[j], 16 * ((k - NBS) // NBS + 1))
                g.dma_gather(
                    sbufs[j][:],
                    w_hbm[:],
                    sidx_sb[:, k * CS : (k + 1) * CS],
                    C,
                    C,
                    E,
                    queue_num=j,  # SWDGE completion sems are queue-locked
                ).then_inc(gsems_s[j], 16)

        # quad write-outs split across the two HWDGE engines (SP=even,
        # ACT=odd chunks); singles land on SP at the end
        def _writer(eng, parity):
            for k in range(parity, QCH, 2):
                j = k % NB
                eng.wait_ge(gsems[j], 16 * (k // NB + 1))
                eng.dma_start(outq_hbm[k], qbufs[j][:]).then_inc(wsems[j], 16)
            for j in range(parity, NB, 2):
                ks = [k for k in range(QCH) if k % NB == j]
                eng.wait_ge(wsems[j], 16 * len(ks))

        @block.sync
        def _(s: bass.BassEngine):
            _writer(s, 0)
            for k in range(SCH):
                j = k % NBS
                s.wait_ge(gsems_s[j], 16 * (k // NBS + 1))
                s.dma_start(outs_hbm[k], sbufs[j][:]).then_inc(wsems_s[j], 16)
            for j in range(NBS):
                ks = [k for k in range(SCH) if k % NBS == j]
                s.wait_ge(wsems_s[j], 16 * len(ks))

        @block.scalar
        def _(sc: bass.BassEngine):
            _writer(sc, 1)

    nc.compile()
    return nc


def _get_compiled():
    global _compiled
    if _compiled is None:
        _compiled = _build()
    return _compiled


def _idx_tile(vals, nch):
    """[nch*C] int16 -> dma_gather layout [128, nch*CS] (i -> partition i%16,
    col chunk*CS + i//16, replicated on the 8 partition groups)."""
    t = vals.reshape(nch, CS, 16).transpose(2, 0, 1).reshape(16, -1)
    return np.tile(t, (8, 1))


def _run(x, W, b, trace=False):
    x = np.asarray(x)
    W = np.asarray(W, dtype=np.float32)
    b = np.asarray(b, dtype=np.float32)
    orig_shape = x.shape
    xf = np.ascontiguousarray(x).reshape(-1).astype(np.int64)
    n_tok = xf.shape[0]

    table = W.T + b  # bias folded in exactly (fp32 add, matches reference)

    order = np.argsort(xf, kind="stable")
    counts = np.bincount(xf[order] // SHARD, minlength=N_CORES)
    starts = np.concatenate(([0], np.cumsum(counts)))[:N_CORES]

    in_maps = []
    host_jobs = []
    for c in range(N_CORES):
        n_c = int(counts[c])
        pos_c = order[starts[c] : starts[c] + n_c]
        extra_pos = None
        if n_c > N_PAD:  # statistically never; exact host fallback
            extra_pos = pos_c[N_PAD:]
            pos_c = pos_c[:N_PAD]
            n_c = N_PAD
        loc = (xf[pos_c] - c * SHARD).astype(np.int32)
        pad = np.full(N_PAD, loc[-1] if n_c else 0, dtype=np.int32)
        pad[:n_c] = loc  # tail padding keeps the array sorted

        base = np.minimum(pad[0::BLK], SHARD - BLK)
        sub = pad.reshape(-1, BLK) - base[:, None]
        ok = (sub >= 0) & (sub <= BLK - 1)
        left_j = np.flatnonzero(~ok.reshape(-1))  # token slots needing singles
        left_j = left_j[left_j < n_c]

        qvals = np.zeros(QCH * C, dtype=np.int16)
        qvals[:N_QUAD] = base.astype(np.int16)
        svals = np.zeros(SCH * C, dtype=np.int16)
        ns = min(len(left_j), SCH * C)
        svals[:ns] = pad[left_j[:ns]].astype(np.int16)

        in_maps.append(
            {
                "w": np.ascontiguousarray(table[c * SHARD : (c + 1) * SHARD]),
                "qidx": _idx_tile(qvals, QCH),
                "sidx": _idx_tile(svals, SCH),
            }
        )
        host_jobs.append((pos_c, n_c, sub, left_j, ns, extra_pos))

    nc = _get_compiled()
    br = run_bass_kernel_spmd(nc, in_maps, core_ids=list(range(N_CORES)), trace=trace)

    out_full = np.empty((n_tok, E), dtype=np.float32)
    tok_quad = np.arange(N_PAD) // BLK
    for c in range(N_CORES):
        pos_c, n_c, sub, left_j, ns, extra_pos = host_jobs[c]
        # quad block i -> [chunk i//1024, partition i%128, slot (i%1024)//128]
        qdev = (
            br.results[c]["outq"]
            .reshape(QCH, 128, F, QE)
            .transpose(0, 2, 1, 3)
            .reshape(QCH * C, BLK, E)
        )
        subf = np.clip(sub.reshape(-1), 0, BLK - 1)
        rows = qdev[tok_quad, subf]  # [N_PAD, E]
        if ns:
            sdev = (
                br.results[c]["outs"]
                .reshape(SCH, 128, F, E)
                .transpose(0, 2, 1, 3)
                .reshape(SCH * C, E)
            )
            rows[left_j[:ns]] = sdev[:ns]
        if len(left_j) > ns:  # singles overflow: exact host fallback
            j = left_j[ns:]
            rows[j] = table[xf[pos_c[j]]]
        out_full[pos_c] = rows[:n_c]
        if extra_pos is not None:
            out_full[extra_pos] = table[xf[extra_pos]]

    return out_full.reshape(*orig_shape, E), br


def kernel(x, W, b):
    out, _ = _run(x, W, b, trace=False)
    return out


# revision 17
# speedup vs baseline: 1.2770x; 1.0178x over previous
"""Embedding lookup (gather rows of W.T by index, + bias) on 8 TRN2 cores.

Strategy: vocab-sharded ("row-parallel") embedding. The bias is folded into
the table on the host (out = (W.T + b)[x], exactly). Each core owns a
12500-row shard of the 100000-row table; the host routes each token index to
its owning core via one argsort (grouping by shard AND sorting ascending
within it), the device does the data movement, and the host applies the
inverse permutation to assemble the full [4096, 200, 64] output.

Device kernel (SPMD on 8 cores, identical program), built around the
gpsimd dma_gather primitive (SWDGE: one DMA descriptor per index):

- BLOCK pass: sorted indices have ~8x multiplicity, so BLK=8 consecutive
  sorted tokens almost always fall within an 8-row window of the table. One
  2048 B descriptor (8 overlapping rows, elem_step=64 elems, elem_size=512)
  serves 8 tokens at SDMA line rate, amortizing the ~200 ns HBM random-read
  latency per descriptor that dominates at 256 B. The host picks each
  block's base row and later slices each token's row out of its block (pure
  permutation).
- SINGLES pass: the rare tokens whose row falls outside their block's 8-row
  window (none at this multiplicity, but kept for robustness) are gathered
  separately at 256 B.
- Chunks of 1024 indices (single_packet dma_gather caps at 64 descs/lane),
  rotating over 4 SWDGE queues (one descriptor ring each) and 8 SBUF
  buffers; the two HWDGE engines (sync/scalar) stream gathered buffers to
  HBM, overlapped with subsequent gathers.
"""

import contextlib

import numpy as np

import concourse.bass as bass
import concourse.bacc as bacc
import concourse.mybir as mybir
from concourse.library_config import mlp
from concourse.bass_utils import run_bass_kernel_spmd

VOCAB = 100000
E = 64                    # embedding dim; 256 B rows
BLK = 8                   # tokens (and table rows) per gathered block
QE = BLK * E              # block: 8 rows = 2048 B
N_CORES = 8
SHARD = VOCAB // N_CORES  # 12500 rows per core (< int16 max)
C = 1024                  # indices per dma_gather (single_packet limit)
N_PAD = 104448            # padded tokens per core (max bucket 102771 @ seed)
N_QUAD = N_PAD // BLK     # 13056 blocks
QCH = 13                  # block chunks (13312 slots, tail padded)
SCH = 4                   # singles chunks (4096 slots for quad violators)
F = C // 128              # free slots per chunk
NB = 8                    # rotating quad buffers
NBS = 2                   # rotating singles buffers
NQ = 4                    # SWDGE queues
CS = C // 16              # idx-tile columns per chunk
QIDX_PIECES = [0, 2, 7, QCH]  # block idx tile load pieces (chunk bounds)

_compiled = None


def _build():
    nc = bacc.Bacc("TRN2", num_swdge_queues=NQ)
    w_hbm = nc.dram_tensor("w", [SHARD, E], mybir.dt.float32, kind="ExternalInput")
    qidx_hbm = nc.dram_tensor(
        "qidx", [128, QCH * CS], mybir.dt.int16, kind="ExternalInput"
    )
    sidx_hbm = nc.dram_tensor(
        "sidx", [128, SCH * CS], mybir.dt.int16, kind="ExternalInput"
    )
    outq_hbm = nc.dram_tensor(
        "outq", [QCH, 128, F * QE], mybir.dt.float32, kind="ExternalOutput"
    )
    outs_hbm = nc.dram_tensor(
        "outs", [SCH, 128, F * E], mybir.dt.float32, kind="ExternalOutput"
    )

    # overlapping view of the table: "row" r = elements [r*64, r*64 + 256)
    w_quad = w_hbm[:].copy()
    w_quad.ap[0] = (E, SHARD - (BLK - 1))
    w_quad.ap[1] = (1, QE)

    with contextlib.ExitStack() as stack:
        block = stack.enter_context(nc.Block())
        qidx_sb = stack.enter_context(
            nc.sbuf_tensor("qidx_sb", [128, QCH * CS], mybir.dt.int16)
        )
        sidx_sb = stack.enter_context(
            nc.sbuf_tensor("sidx_sb", [128, SCH * CS], mybir.dt.int16)
        )
        qbufs = [
            stack.enter_context(
                nc.sbuf_tensor(f"qbuf{j}", [128, F, QE], mybir.dt.float32)
            )
            for j in range(NB)
        ]
        sbufs = [
            stack.enter_context(
                nc.sbuf_tensor(f"sbuf{j}", [128, F, E], mybir.dt.float32)
            )
            for j in range(NBS)
        ]
        isems = [stack.enter_context(nc.semaphore(f"isem{p}")) for p in range(3)]
        ssem = stack.enter_context(nc.semaphore("ssem"))
        gsems = [stack.enter_context(nc.semaphore(f"g{j}")) for j in range(NB)]
        wsems = [stack.enter_context(nc.semaphore(f"ws{j}")) for j in range(NB)]
        gsems_s = [stack.enter_context(nc.semaphore(f"gs{j}")) for j in range(NBS)]
        wsems_s = [stack.enter_context(nc.semaphore(f"wss{j}")) for j in range(NBS)]

        @block.gpsimd
        def _(g: bass.BassGpSimd):
            # idx loads via SWDGE (deterministic +16/DMA); drains overlap the
            # library load that follows
            for p in range(3):
                a, b = QIDX_PIECES[p] * CS, QIDX_PIECES[p + 1] * CS
                g.dma_start(qidx_sb[:, a:b], qidx_hbm[:, a:b]).then_inc(isems[p], 16)
            g.dma_start(sidx_sb[:], sidx_hbm[:]).then_inc(ssem, 16)
            g.load_library(mlp)
            for k in range(QCH):
                j = k % NB
                if k in QIDX_PIECES[:3]:
                    g.wait_ge(isems[QIDX_PIECES.index(k)], 16)
                if k >= NB:
                    g.wait_ge(wsems[j], 16 * ((k - NB) // NB + 1))
                g.dma_gather(
                    qbufs[j][:],
                    w_quad,
                    qidx_sb[:, k * CS : (k + 1) * CS],
                    C,
                    C,
                    QE,
                    elem_step=E,
                    # queues 2/3 only: SWDGE contexts 0/1 share SDMA internal
                    # queues with the two HWDGE write rings and drain at half
                    # rate when writes are active
                    queue_num=2 + (k % 2),
                ).then_inc(gsems[j], 16)
            g.wait_ge(ssem, 16)
            for k in range(SCH):
                j = k % NBS
                if k >= NBS:
                    g.wait_ge(wsems_s[j], 16 * ((k - NBS) // NBS + 1))
                g.dma_gather(
                    sbufs[j][:],
                    w_hbm[:],
                    sidx_sb[:, k * CS : (k + 1) * CS],
                    C,
                    C,
                    E,
                    queue_num=2 + j,  # SWDGE completion sems are queue-locked
                ).then_inc(gsems_s[j], 16)

        # quad write-outs split across the two HWDGE engines (SP=even,
        # ACT=odd chunks); singles land on SP at the end
        def _writer(eng, parity):
            for k in range(parity, QCH, 2):
                j = k % NB
                eng.wait_ge(gsems[j], 16 * (k // NB + 1))
                eng.dma_start(outq_hbm[k], qbufs[j][:]).then_inc(wsems[j], 16)
            for j in range(parity, NB, 2):
                ks = [k for k in range(QCH) if k % NB == j]
                eng.wait_ge(wsems[j], 16 * len(ks))

        @block.sync
        def _(s: bass.BassEngine):
            _writer(s, 0)
            for k in range(SCH):
                j = k % NBS
                s.wait_ge(gsems_s[j], 16 * (k // NBS + 1))
                s.dma_start(outs_hbm[k], sbufs[j][:]).then_inc(wsems_s[j], 16)
            for j in range(NBS):
                ks = [k for k in range(SCH) if k % NBS == j]
                s.wait_ge(wsems_s[j], 16 * len(ks))

        @block.scalar
        def _(sc: bass.BassEngine):
            _writer(sc, 1)

    nc.compile()
    return nc


def _get_compiled():
    global _compiled
    if _compiled is None:
        _compiled = _build()
    return _compiled


def _idx_tile(vals, nch):
    """[nch*C] int16 -> dma_gather layout [128, nch*CS] (i -> partition i%16,
    col chunk*CS + i//16, replicated on the 8 partition groups)."""
    t = vals.reshape(nch, CS, 16).transpose(2, 0, 1).reshape(16, -1)
    return np.tile(t, (8, 1))


def _run(x, W, b, trace=False):
    x = np.asarray(x)
    W = np.asarray(W, dtype=np.float32)
    b = np.asarray(b, dtype=np.float32)
    orig_shape = x.shape
    xf = np.ascontiguousarray(x).reshape(-1).astype(np.int64)
    n_tok = xf.shape[0]

    table = W.T + b  # bias folded in exactly (fp32 add, matches reference)

    order = np.argsort(xf, kind="stable")
    counts = np.bincount(xf[order] // SHARD, minlength=N_CORES)
    starts = np.concatenate(([0], np.cumsum(counts)))[:N_CORES]

    in_maps = []
    host_jobs = []
    for c in range(N_CORES):
        n_c = int(counts[c])
        pos_c = order[starts[c] : starts[c] + n_c]
        extra_pos = None
        if n_c > N_PAD:  # statistically never; exact host fallback
            extra_pos = pos_c[N_PAD:]
            pos_c = pos_c[:N_PAD]
            n_c = N_PAD
        loc = (xf[pos_c] - c * SHARD).astype(np.int32)
        pad = np.full(N_PAD, loc[-1] if n_c else 0, dtype=np.int32)
        pad[:n_c] = loc  # tail padding keeps the array sorted

        base = np.minimum(pad[0::BLK], SHARD - BLK)
        sub = pad.reshape(-1, BLK) - base[:, None]
        ok = (sub >= 0) & (sub <= BLK - 1)
        left_j = np.flatnonzero(~ok.reshape(-1))  # token slots needing singles
        left_j = left_j[left_j < n_c]

        qvals = np.zeros(QCH * C, dtype=np.int16)
        qvals[:N_QUAD] = base.astype(np.int16)
        svals = np.zeros(SCH * C, dtype=np.int16)
        ns = min(len(left_j), SCH * C)
        svals[:ns] = pad[left_j[:ns]].astype(np.int16)

        in_maps.append(
            {
                "w": np.ascontiguousarray(table[c * SHARD : (c + 1) * SHARD]),
                "qidx": _idx_tile(qvals, QCH),
                "sidx": _idx_tile(svals, SCH),
            }
        )
        host_jobs.append((pos_c, n_c, sub, left_j, ns, extra_pos))

    nc = _get_compiled()
    br = run_bass_kernel_spmd(nc, in_maps, core_ids=list(range(N_CORES)), trace=trace)

    out_full = np.empty((n_tok, E), dtype=np.float32)
    tok_quad = np.arange(N_PAD) // BLK
    for c in range(N_CORES):
        pos_c, n_c, sub, left_j, ns, extra_pos = host_jobs[c]
        # quad block i -> [chunk i//1024, partition i%128, slot (i%1024)//128]
        qdev = (
            br.results[c]["outq"]
            .reshape(QCH, 128, F, QE)
            .transpose(0, 2, 1, 3)
            .reshape(QCH * C, BLK, E)
        )
        subf = np.clip(sub.reshape(-1), 0, BLK - 1)
        rows = qdev[tok_quad, subf]  # [N_PAD, E]
        if ns:
            sdev = (
                br.results[c]["outs"]
                .reshape(SCH, 128, F, E)
                .transpose(0, 2, 1, 3)
                .reshape(SCH * C, E)
            )
            rows[left_j[:ns]] = sdev[:ns]
        if len(left_j) > ns:  # singles overflow: exact host fallback
            j = left_j[ns:]
            rows[j] = table[xf[pos_c[j]]]
        out_full[pos_c] = rows[:n_c]
        if extra_pos is not None:
            out_full[extra_pos] = table[xf[extra_pos]]

    return out_full.reshape(*orig_shape, E), br


def kernel(x, W, b):
    out, _ = _run(x, W, b, trace=False)
    return out
